# revision 11
# baseline (speedup 1.0000x reference)
"""BiUTE kernel for Trainium2, 8-core data-parallel over batch.

Math (per batch element b, T=128, N=12, D=1024, F=2D=2048):
  u = Wq.sum(0)                                  [D]
  w[t,n]  = sum_d feat[t,n,d] * u[d]             [T,N]
  g[t,d]  = sum_n w[t,n] * feat[t,n,d]           [T,D]
  f[t,d]  = max_n feat[t,n,d]                    [T,D]
  n = [g | f]                                    [T,F]
  tb = n @ Wtb.T ; pb = n @ Wpb.T ; gb = n @ Wgb.T
  sb = (tb @ pb.T) * scale ; out_b = (sb*lower) @ gb
  (same for 'after' branch with upper mask)
  out = n + out_b + out_a                        [T,F]

Sharding: B=16 split 2 per core across 8 cores; weights replicated.
On-chip: fp16 matmul operands, fp32 accumulation/output.
"""

import numpy as np

import concourse.bass as bass
import concourse.mybir as mybir
import concourse.tile as tile
from concourse import bacc
from concourse.bass_utils import run_bass_kernel_spmd

F32 = mybir.dt.float32
F16 = mybir.dt.float16

B, T, NP, D = 16, 128, 12, 1024
F = 2 * D                      # 2048
NB = 2                         # batch elements per core
NCORES = 8
TN = T * NP                    # 1536 flattened (t, n) rows
NCH = TN // 128                # 12 feature chunks of 128 rows
NFC = F // 128                 # 16 f-chunks of nT
SCALE = 1.0 / float(np.sqrt(F))

_CACHE = {}
DEBUG = False
STAGE = 0  # 0=full, 1=phaseA, 2=+pass1, 3=+pass2, 4=+pass3 (before branch only)


def _build():
    nc = bacc.Bacc("TRN2", target_bir_lowering=False, debug=False)

    featd = nc.dram_tensor("feat", [NB, TN, D], F16, kind="ExternalInput")
    ud = nc.dram_tensor("u", [1, D], F16, kind="ExternalInput")
    mbd = nc.dram_tensor("maskb", [T, T], F32, kind="ExternalInput")
    mad = nc.dram_tensor("maska", [T, T], F32, kind="ExternalInput")
    identd = nc.dram_tensor("ident", [128, 128], F32, kind="ExternalInput")
    wtpbd = nc.dram_tensor("wtp_b", [F, F], F16, kind="ExternalInput")
    wgbd = nc.dram_tensor("wg_b", [F, F], F16, kind="ExternalInput")
    wtpad = nc.dram_tensor("wtp_a", [F, F], F16, kind="ExternalInput")
    wgad = nc.dram_tensor("wg_a", [F, F], F16, kind="ExternalInput")
    outd = nc.dram_tensor("out", [NB, T, F], F32, kind="ExternalOutput")
    if DEBUG:
        dbg_n = nc.dram_tensor("dbg_n", [NB, T, F], F32, kind="ExternalOutput")
        dbg_nT = nc.dram_tensor("dbg_nT", [128, NFC, NB * T], F16, kind="ExternalOutput")
        dbg_tp2 = nc.dram_tensor("dbg_tp2", [2, 128, 16, 2 * T], F16, kind="ExternalOutput")
        dbg_gb = nc.dram_tensor("dbg_gb", [2, NB, T, F], F16, kind="ExternalOutput")
        dbg_sbm = nc.dram_tensor("dbg_sbm", [2, NB, T, T], F16, kind="ExternalOutput")

    with tile.TileContext(nc) as tc:
        with (
            tc.tile_pool(name="consts", bufs=1) as consts,
            tc.tile_pool(name="npool", bufs=1) as npool,
            tc.tile_pool(name="ntpool", bufs=1) as ntpool,
        ):
            u_sb = consts.tile([128, D], F16)
            nc.gpsimd.dma_start(out=u_sb[:], in_=ud[:].to_broadcast((128, D)))
            mb_sb = consts.tile([T, T], F32)
            nc.sync.dma_start(out=mb_sb[:], in_=mbd[:])
            ma_sb = consts.tile([T, T], F32)
            nc.sync.dma_start(out=ma_sb[:], in_=mad[:])
            ident = consts.tile([128, 128], F32)
            nc.sync.dma_start(out=ident[:], in_=identd[:])

            # n (fp32) per batch element; nT (fp16) shared [128, fc, 2*T]
            n_sb = [npool.tile([T, F], F32, tag=f"n{b}", name=f"n{b}") for b in range(NB)]
            nT = ntpool.tile([128, NFC, NB * T], F16)

            # ---------------- Phase A: w, f, g, n, nT ----------------
            # feat loaded t-major: tile [t=128, n=12, d=1024].
            with (
                tc.tile_pool(name="featp", bufs=2) as featp,
                tc.tile_pool(name="aw", bufs=2) as awp,
                tc.tile_pool(name="psAt", bufs=2, space="PSUM") as psAt,
            ):
                for b in range(NB):
                    feat = featp.tile([T, NP, D], F16, tag="feat")
                    # 4 DMAs of 32 t-rows each for pipelining
                    for q in range(4):
                        nc.sync.dma_start(
                            out=feat[32 * q : 32 * (q + 1)],
                            in_=featd[b, 384 * q : 384 * (q + 1), :].rearrange(
                                "(p c) d -> p c d", c=NP
                            ),
                        )
                    # w[t,n] = sum_d feat*u  (fp32 accumulate)
                    # (tensor_tensor_reduce crashes TRN2 here; use mul+reduce)
                    wvec = awp.tile([T, NP], F32, tag="wvec")
                    scr = awp.tile([T, D], F16, tag="scr")
                    for c in range(NP):
                        nc.vector.tensor_mul(scr[:], feat[:, c, :], u_sb[:])
                        nc.vector.tensor_reduce(
                            out=wvec[:, c : c + 1],
                            in_=scr[:],
                            axis=mybir.AxisListType.X,
                            op=mybir.AluOpType.add,
                        )
                    # g[t,d] = sum_n w[t,n]*feat[t,n,d], accumulated in n_sb
                    nc.vector.tensor_scalar_mul(
                        n_sb[b][:, :D], feat[:, 0, :], wvec[:, 0:1]
                    )
                    for c in range(1, NP):
                        nc.vector.scalar_tensor_tensor(
                            out=n_sb[b][:, :D],
                            in0=feat[:, c, :],
                            scalar=wvec[:, c : c + 1],
                            in1=n_sb[b][:, :D],
                            op0=mybir.AluOpType.mult,
                            op1=mybir.AluOpType.add,
                        )
                    # f = max_n feat  -> fp16 tmp -> n[:, D:]
                    facc = awp.tile([T, D], F16, tag="facc")
                    nc.vector.tensor_max(facc[:], feat[:, 0, :], feat[:, 1, :])
                    for c in range(2, NP):
                        nc.vector.tensor_max(facc[:], facc[:], feat[:, c, :])
                    nc.vector.tensor_copy(n_sb[b][:, D:], facc[:])
                    # nT via PE transpose of n (fp32 in, fp16 out-cast)
                    for fc in range(NFC):
                        pt = psAt.tile([128, 128], F32, tag="pt")
                        nc.tensor.transpose(
                            pt[:], n_sb[b][:, 128 * fc : 128 * (fc + 1)], ident[:]
                        )
                        nc.vector.tensor_copy(
                            nT[:, fc, T * b : T * (b + 1)], pt[:]
                        )

            if DEBUG:
                for b in range(NB):
                    nc.sync.dma_start(out=dbg_n[b], in_=n_sb[b][:])
                nc.sync.dma_start(out=dbg_nT[:], in_=nT[:])

            # ---------------- Phases B/C: the two branches ----------------
            branches = ((wtpbd, wgbd, mb_sb), (wtpad, wgad, ma_sb))
            if STAGE == 1:
                branches = ()
            elif STAGE in (2, 3, 4):
                branches = branches[:1]
            for ibr, (wtpd, wgd, mask_sb) in enumerate(branches):
                with (
                    tc.tile_pool(name="wres", bufs=2) as wsp,
                    tc.tile_pool(name="drains", bufs=1) as drp,
                    tc.tile_pool(name="sbp", bufs=2) as sbp,
                ):
                    # ---- pass 1: tbT/pbT, [e-sub, proj*echunk, t2] ----
                    # Weights resident; one full-bank PSUM accumulation group
                    # per e-chunk (start=True zeroes a whole bank).
                    with tc.tile_pool(name="ps1", bufs=2, space="PSUM") as ps1p:
                        wt_all = wsp.tile([128, NFC, F], F16, tag="w")
                        for q in range(8):  # 2 f-chunks per DMA
                            nc.sync.dma_start(
                                out=wt_all[:, 2 * q : 2 * (q + 1), :],
                                in_=wtpd[256 * q : 256 * (q + 1), :].rearrange(
                                    "(c p) e -> p c e", p=128
                                ),
                            )
                        tp2 = drp.tile([128, 16, 2 * T], F16, tag="tp2")
                        for e16 in range(16):
                            p1 = ps1p.tile([128, 2 * T], F32, tag="p1")
                            for fc in range(NFC):
                                nc.tensor.matmul(
                                    p1[:],
                                    wt_all[:, fc, 128 * e16 : 128 * (e16 + 1)],
                                    nT[:, fc, :],
                                    start=(fc == 0),
                                    stop=(fc == NFC - 1),
                                )
                            nc.vector.tensor_copy(tp2[:, e16, :], p1[:])
                        if DEBUG:
                            nc.sync.dma_start(out=dbg_tp2[ibr], in_=tp2[:])

                    # ---- pass 2: gb natural per b ----
                    if STAGE == 2:
                        continue
                    with tc.tile_pool(name="ps2", bufs=1, space="PSUM") as ps2p:
                        psg = [
                            ps2p.tile(
                                [128, 4, 512], F32, tag=f"psg{b}", name=f"psg{b}"
                            )
                            for b in range(NB)
                        ]
                        for q in range(8):
                            wg = wsp.tile([128, 2, F], F16, tag="wg")
                            nc.sync.dma_start(
                                out=wg[:],
                                in_=wgd[256 * q : 256 * (q + 1), :].rearrange(
                                    "(c p) e -> p c e", p=128
                                ),
                            )
                            for s in range(2):
                                fc = 2 * q + s
                                for b in range(NB):
                                    for h4 in range(4):
                                        nc.tensor.matmul(
                                            psg[b][:, h4, :],
                                            nT[:, fc, T * b : T * (b + 1)],
                                            wg[:, s, 512 * h4 : 512 * (h4 + 1)],
                                            start=(fc == 0),
                                            stop=(fc == NFC - 1),
                                        )
                        gb16 = [
                            drp.tile([T, F], F16, tag=f"gb{b}", name=f"gb{b}")
                            for b in range(NB)
                        ]
                        for b in range(NB):
                            for h4 in range(4):
                                nc.vector.tensor_copy(
                                    gb16[b][:, 512 * h4 : 512 * (h4 + 1)],
                                    psg[b][:, h4, :],
                                )
                        if DEBUG:
                            for b in range(NB):
                                nc.sync.dma_start(out=dbg_gb[ibr, b], in_=gb16[b][:])

                    # ---- pass 3: sbT, mask, out accumulation ----
                    if STAGE == 3:
                        continue
                    with (
                        tc.tile_pool(name="ps3", bufs=2, space="PSUM") as ps3p,
                        tc.tile_pool(name="ps4", bufs=2, space="PSUM") as ps4p,
                    ):
                        for b in range(NB):
                            psb = ps3p.tile([T, T], F32, tag="psb")
                            for ec in range(8):
                                nc.tensor.matmul(
                                    psb[:],
                                    tp2[:, 8 + ec, T * b : T * (b + 1)],
                                    tp2[:, ec, T * b : T * (b + 1)],
                                    start=(ec == 0),
                                    stop=(ec == 7),
                                )
                            sbm = sbp.tile([T, T], F16, tag="sbm")
                            nc.vector.scalar_tensor_tensor(
                                out=sbm[:],
                                in0=psb[:],
                                scalar=1.0,
                                in1=mask_sb[:],
                                op0=mybir.AluOpType.mult,
                                op1=mybir.AluOpType.mult,
                            )
                            if DEBUG:
                                nc.sync.dma_start(out=dbg_sbm[ibr, b], in_=sbm[:])
                            for h4 in range(4):
                                po = ps4p.tile([T, 512], F32, tag="po")
                                nc.tensor.matmul(
                                    po[:],
                                    sbm[:],
                                    gb16[b][:, 512 * h4 : 512 * (h4 + 1)],
                                    start=True,
                                    stop=True,
                                )
                                nc.vector.tensor_add(
                                    n_sb[b][:, 512 * h4 : 512 * (h4 + 1)],
                                    n_sb[b][:, 512 * h4 : 512 * (h4 + 1)],
                                    po[:],
                                )

            for b in range(NB):
                nc.sync.dma_start(out=outd[b], in_=n_sb[b][:])

    nc.compile()
    return nc


def _host_prep(features, Wq, Wtb, Wpb, Wgb, Wta, Wpa, Wga):
    f32 = np.float32
    f16 = np.float16
    feat = np.ascontiguousarray(np.asarray(features, f32)).reshape(B, TN, D)
    u = np.asarray(Wq, f32).sum(axis=0)[None, :]

    def wt(w):  # [e, f] -> [f, e] fp16 contiguous
        return np.ascontiguousarray(np.asarray(w, f32).T.astype(f16))

    wtp_b = np.concatenate([wt(Wtb), wt(Wpb)], axis=1)
    wtp_a = np.concatenate([wt(Wta), wt(Wpa)], axis=1)
    wg_b = wt(Wgb)
    wg_a = wt(Wga)

    idx = np.arange(T)
    maskb = (SCALE * (idx[None, :] > idx[:, None])).astype(f32)  # [j, i]
    maska = (SCALE * (idx[None, :] < idx[:, None])).astype(f32)
    ident = np.eye(128, dtype=f32)

    shared = {
        "u": u.astype(f16),
        "maskb": maskb,
        "maska": maska,
        "ident": ident,
        "wtp_b": wtp_b,
        "wg_b": wg_b,
        "wtp_a": wtp_a,
        "wg_a": wg_a,
    }
    feat16 = feat.astype(f16).reshape(NCORES, NB, TN, D)
    return shared, feat16


def kernel(**inputs) -> np.ndarray:
    if "nc" not in _CACHE:
        _CACHE["nc"] = _build()
    nc = _CACHE["nc"]

    shared, feat16 = _host_prep(**inputs)
    in_maps = [dict(shared, feat=feat16[c]) for c in range(NCORES)]
    res = run_bass_kernel_spmd(nc, in_maps, core_ids=list(range(NCORES)))
    out = np.stack([res.results[c]["out"] for c in range(NCORES)], axis=0)
    return out.reshape(B, T, F).astype(np.float32)


# revision 12
# speedup vs baseline: 1.2650x; 1.2650x over previous
"""BiUTE kernel for Trainium2, 8-core data-parallel over batch.

Math (per batch element b, T=128, N=12, D=1024, F=2D=2048):
  u = Wq.sum(0)                                  [D]
  w[t,n]  = sum_d feat[t,n,d] * u[d]             [T,N]
  g[t,d]  = sum_n w[t,n] * feat[t,n,d]           [T,D]
  f[t,d]  = max_n feat[t,n,d]                    [T,D]
  n = [g | f]                                    [T,F]
  tb = n @ Wtb.T ; pb = n @ Wpb.T ; gb = n @ Wgb.T
  sb = (tb @ pb.T) * scale ; out_b = (sb*lower) @ gb
  (same for 'after' branch with upper mask)
  out = n + out_b + out_a                        [T,F]

Sharding: B=16 split 2 per core across 8 cores; weights replicated.
On-chip: fp16 matmul operands, fp32 accumulation/output. Weights stream
through two 4.2MB SBUF slots so prefetch overlaps phase A.
"""

import numpy as np

import concourse.mybir as mybir
import concourse.tile as tile
from concourse import bacc
from concourse.bass_utils import run_bass_kernel_spmd

F32 = mybir.dt.float32
F16 = mybir.dt.float16

B, T, NP, D = 16, 128, 12, 1024
F = 2 * D                      # 2048
NB = 2                         # batch elements per core
NCORES = 8
TN = T * NP                    # 1536 flattened (t, n) rows
NFC = F // 128                 # 16 f-chunks of nT
SCALE = 1.0 / float(np.sqrt(F))

_CACHE = {}


def _build():
    nc = bacc.Bacc("TRN2", target_bir_lowering=False, debug=False)
    mult = mybir.AluOpType.mult
    add = mybir.AluOpType.add

    featd = nc.dram_tensor("feat", [NB, TN, D], F16, kind="ExternalInput")
    ud = nc.dram_tensor("u", [1, D], F16, kind="ExternalInput")
    mbd = nc.dram_tensor("maskb", [T, T], F32, kind="ExternalInput")
    mad = nc.dram_tensor("maska", [T, T], F32, kind="ExternalInput")
    identd = nc.dram_tensor("ident", [128, 128], F32, kind="ExternalInput")
    wtpbd = nc.dram_tensor("wtp_b", [F, F], F16, kind="ExternalInput")
    wgbd = nc.dram_tensor("wg_b", [F, F], F16, kind="ExternalInput")
    wtpad = nc.dram_tensor("wtp_a", [F, F], F16, kind="ExternalInput")
    wgad = nc.dram_tensor("wg_a", [F, F], F16, kind="ExternalInput")
    outd = nc.dram_tensor("out", [NB, T, F], F32, kind="ExternalOutput")

    with tile.TileContext(nc) as tc:
        with (
            tc.tile_pool(name="consts", bufs=1) as consts,
            tc.tile_pool(name="npool", bufs=1) as npool,
            tc.tile_pool(name="ntpool", bufs=1) as ntpool,
            tc.tile_pool(name="wres", bufs=2) as wsp,
            tc.tile_pool(name="drains", bufs=1) as drp,
        ):
            u_sb = consts.tile([128, D], F16)
            nc.gpsimd.dma_start(out=u_sb[:], in_=ud[:].to_broadcast((128, D)))
            mb_sb = consts.tile([T, T], F32)
            nc.sync.dma_start(out=mb_sb[:], in_=mbd[:])
            ma_sb = consts.tile([T, T], F32)
            nc.sync.dma_start(out=ma_sb[:], in_=mad[:])
            ident = consts.tile([128, 128], F32)
            nc.sync.dma_start(out=ident[:], in_=identd[:])

            # n (fp32) per batch element; nT (fp16) shared [128, fc, 2*T]
            n_sb = [
                npool.tile([T, F], F32, tag=f"n{b}", name=f"n{b}")
                for b in range(NB)
            ]
            nT = ntpool.tile([128, NFC, NB * T], F16)

            def load_w_half(src, colh):
                """Stream one [2048, 1024] fp16 weight half into a slot."""
                wh = wsp.tile([128, NFC, D], F16, tag="w", name="wh")
                for q in range(8):
                    nc.sync.dma_start(
                        out=wh[:, 2 * q : 2 * (q + 1), :],
                        in_=src[
                            256 * q : 256 * (q + 1), D * colh : D * (colh + 1)
                        ].rearrange("(c p) e -> p c e", p=128),
                    )
                return wh

            # ---------------- Phase A: f, w, g, n, nT ----------------
            with (
                tc.tile_pool(name="featp", bufs=2) as featp,
                tc.tile_pool(name="aw", bufs=2) as awp,
                tc.tile_pool(name="psAt", bufs=2, space="PSUM") as psAt,
            ):
                for b in range(NB):
                    feat = featp.tile([T, NP, D], F16, tag="feat")
                    for q in range(4):
                        nc.sync.dma_start(
                            out=feat[32 * q : 32 * (q + 1)],
                            in_=featd[b, 384 * q : 384 * (q + 1), :].rearrange(
                                "(p c) d -> p c d", c=NP
                            ),
                        )

                    def transp(fc, b=b):
                        pt = psAt.tile([128, 128], F32, tag="pt", name="pt")
                        nc.tensor.transpose(
                            pt[:], n_sb[b][:, 128 * fc : 128 * (fc + 1)],
                            ident[:],
                        )
                        nc.vector.tensor_copy(
                            nT[:, fc, T * b : T * (b + 1)], pt[:]
                        )

                    # f = max_n feat (first, so its transposes start early)
                    facc = awp.tile([T, D], F16, tag="facc")
                    nc.vector.tensor_max(facc[:], feat[:, 0, :], feat[:, 1, :])
                    for c in range(2, NP):
                        nc.vector.tensor_max(facc[:], facc[:], feat[:, c, :])
                    nc.vector.tensor_copy(n_sb[b][:, D:], facc[:])
                    for fc in range(8, NFC):
                        transp(fc)

                    # w[t,n] = sum_d feat*u (fused product+reduce, fp32 accum)
                    wvec = awp.tile([T, NP], F32, tag="wvec")
                    scr = awp.tile([T, D], F16, tag="scr")
                    for c in range(NP):
                        nc.vector.scalar_tensor_tensor(
                            out=scr[:],
                            in0=feat[:, c, :],
                            scalar=1.0,
                            in1=u_sb[:],
                            op0=mult,
                            op1=mult,
                            accum_out=wvec[:, c : c + 1],
                        )
                    # g[t,d] = sum_n w[t,n]*feat[t,n,d], accumulated in n_sb
                    nc.vector.tensor_scalar_mul(
                        n_sb[b][:, :D], feat[:, 0, :], wvec[:, 0:1]
                    )
                    for c in range(1, NP):
                        nc.vector.scalar_tensor_tensor(
                            out=n_sb[b][:, :D],
                            in0=feat[:, c, :],
                            scalar=wvec[:, c : c + 1],
                            in1=n_sb[b][:, :D],
                            op0=mult,
                            op1=add,
                        )
                    for fc in range(8):
                        transp(fc)

            # ---------------- Phases B/C: the two branches ----------------
            for ibr, (wtpd, wgd, mask_sb) in enumerate(
                ((wtpbd, wgbd, mb_sb), (wtpad, wgad, ma_sb))
            ):
                sfx = f"_{ibr}"
                # ---- gb = n @ Wg.T (natural, per b; starts after nT[b]) ----
                gb16 = [
                    drp.tile([T, F], F16, tag=f"gb{b}", name=f"gb{b}{sfx}")
                    for b in range(NB)
                ]
                for colh in range(2):
                    wgh = load_w_half(wgd, colh)
                    with tc.tile_pool(
                        name="ps2", bufs=1, space="PSUM"
                    ) as ps2p:
                        psg = [
                            ps2p.tile(
                                [128, 2, 512], F32, tag=f"psg{b}",
                                name=f"psg{b}{sfx}{colh}",
                            )
                            for b in range(NB)
                        ]
                        for b in range(NB):
                            for fc in range(NFC):
                                for h2 in range(2):
                                    nc.tensor.matmul(
                                        psg[b][:, h2, :],
                                        nT[:, fc, T * b : T * (b + 1)],
                                        wgh[:, fc, 512 * h2 : 512 * (h2 + 1)],
                                        start=(fc == 0),
                                        stop=(fc == NFC - 1),
                                    )
                            for h2 in range(2):
                                nc.scalar.copy(
                                    gb16[b][
                                        :,
                                        D * colh + 512 * h2 : D * colh
                                        + 512 * (h2 + 1),
                                    ],
                                    psg[b][:, h2, :],
                                )

                # ---- tbT/pbT: [e-sub, proj*echunk, t2] ----
                tp2 = drp.tile([128, 16, 2 * T], F16, tag="tp2",
                               name=f"tp2{sfx}")
                for colh in range(2):  # 0: tb e-cols, 1: pb e-cols
                    wth = load_w_half(wtpd, colh)
                    with tc.tile_pool(
                        name="ps1", bufs=2, space="PSUM"
                    ) as ps1p:
                        for e8 in range(8):
                            p1 = ps1p.tile([128, 2 * T], F32, tag="p1",
                                           name="p1")
                            for fc in range(NFC):
                                nc.tensor.matmul(
                                    p1[:],
                                    wth[:, fc, 128 * e8 : 128 * (e8 + 1)],
                                    nT[:, fc, :],
                                    start=(fc == 0),
                                    stop=(fc == NFC - 1),
                                )
                            nc.vector.tensor_copy(
                                tp2[:, 8 * colh + e8, :], p1[:]
                            )

                # ---- sbT, mask, out accumulation ----
                with (
                    tc.tile_pool(name="sbp", bufs=2) as sbp,
                    tc.tile_pool(name="ps3", bufs=2, space="PSUM") as ps3p,
                    tc.tile_pool(name="ps4", bufs=2, space="PSUM") as ps4p,
                ):
                    for b in range(NB):
                        psb = ps3p.tile([T, T], F32, tag="psb", name="psb")
                        for ec in range(8):
                            nc.tensor.matmul(
                                psb[:],
                                tp2[:, 8 + ec, T * b : T * (b + 1)],
                                tp2[:, ec, T * b : T * (b + 1)],
                                start=(ec == 0),
                                stop=(ec == 7),
                            )
                        sbm = sbp.tile([T, T], F16, tag="sbm", name="sbm")
                        nc.vector.scalar_tensor_tensor(
                            out=sbm[:],
                            in0=psb[:],
                            scalar=1.0,
                            in1=mask_sb[:],
                            op0=mult,
                            op1=mult,
                        )
                        for h4 in range(4):
                            po = ps4p.tile([T, 512], F32, tag="po", name="po")
                            nc.tensor.matmul(
                                po[:],
                                sbm[:],
                                gb16[b][:, 512 * h4 : 512 * (h4 + 1)],
                                start=True,
                                stop=True,
                            )
                            nc.vector.tensor_add(
                                n_sb[b][:, 512 * h4 : 512 * (h4 + 1)],
                                n_sb[b][:, 512 * h4 : 512 * (h4 + 1)],
                                po[:],
                            )

            for b in range(NB):
                nc.sync.dma_start(out=outd[b], in_=n_sb[b][:])

    nc.compile()
    return nc


def _host_prep(features, Wq, Wtb, Wpb, Wgb, Wta, Wpa, Wga):
    f32 = np.float32
    f16 = np.float16
    feat = np.ascontiguousarray(np.asarray(features, f32)).reshape(B, TN, D)
    u = np.asarray(Wq, f32).sum(axis=0)[None, :]

    def wt(w):  # [e, f] -> [f, e] fp16 contiguous
        return np.ascontiguousarray(np.asarray(w, f32).T.astype(f16))

    wtp_b = np.concatenate([wt(Wtb), wt(Wpb)], axis=1)
    wtp_a = np.concatenate([wt(Wta), wt(Wpa)], axis=1)
    wg_b = wt(Wgb)
    wg_a = wt(Wga)

    idx = np.arange(T)
    maskb = (SCALE * (idx[None, :] > idx[:, None])).astype(f32)  # [j, i]
    maska = (SCALE * (idx[None, :] < idx[:, None])).astype(f32)
    ident = np.eye(128, dtype=f32)

    shared = {
        "u": u.astype(f16),
        "maskb": maskb,
        "maska": maska,
        "ident": ident,
        "wtp_b": wtp_b,
        "wg_b": wg_b,
        "wtp_a": wtp_a,
        "wg_a": wg_a,
    }
    feat16 = feat.astype(f16).reshape(NCORES, NB, TN, D)
    return shared, feat16


def kernel(**inputs) -> np.ndarray:
    if "nc" not in _CACHE:
        _CACHE["nc"] = _build()
    nc = _CACHE["nc"]

    shared, feat16 = _host_prep(**inputs)
    in_maps = [dict(shared, feat=feat16[c]) for c in range(NCORES)]
    res = run_bass_kernel_spmd(nc, in_maps, core_ids=list(range(NCORES)))
    out = np.stack([res.results[c]["out"] for c in range(NCORES)], axis=0)
    return out.reshape(B, T, F).astype(np.float32)


# revision 13
# speedup vs baseline: 1.2761x; 1.0088x over previous
"""BiUTE kernel for Trainium2, 8-core data-parallel over batch.

Math (per batch element b, T=128, N=12, D=1024, F=2D=2048):
  u = Wq.sum(0)                                  [D]
  w[t,n]  = sum_d feat[t,n,d] * u[d]             [T,N]
  g[t,d]  = sum_n w[t,n] * feat[t,n,d]           [T,D]
  f[t,d]  = max_n feat[t,n,d]                    [T,D]
  n = [g | f]                                    [T,F]
  tb = n @ Wtb.T ; pb = n @ Wpb.T ; gb = n @ Wgb.T
  sb = (tb @ pb.T) * scale ; out_b = (sb*lower) @ gb
  (same for 'after' branch with upper mask)
  out = n + out_b + out_a                        [T,F]

Sharding: B=16 split 2 per core across 8 cores; weights replicated.
On-chip: fp16 matmul operands, fp32 accumulation/output. Weights stream
through two 4.2MB SBUF slots so prefetch overlaps phase A.
"""

import numpy as np

import concourse.mybir as mybir
import concourse.tile as tile
from concourse import bacc
from concourse.bass_utils import run_bass_kernel_spmd

F32 = mybir.dt.float32
F16 = mybir.dt.float16

B, T, NP, D = 16, 128, 12, 1024
F = 2 * D                      # 2048
FC_ORDER = list(range(8, 16)) + list(range(8))  # f-half of n first
NB = 2                         # batch elements per core
NCORES = 8
TN = T * NP                    # 1536 flattened (t, n) rows
NFC = F // 128                 # 16 f-chunks of nT
SCALE = 1.0 / float(np.sqrt(F))

_CACHE = {}


def _build():
    nc = bacc.Bacc("TRN2", target_bir_lowering=False, debug=False)
    mult = mybir.AluOpType.mult
    add = mybir.AluOpType.add

    featd = nc.dram_tensor("feat", [NB, TN, D], F16, kind="ExternalInput")
    ud = nc.dram_tensor("u", [1, D], F16, kind="ExternalInput")
    mbd = nc.dram_tensor("maskb", [T, T], F32, kind="ExternalInput")
    mad = nc.dram_tensor("maska", [T, T], F32, kind="ExternalInput")
    identd = nc.dram_tensor("ident", [128, 128], F32, kind="ExternalInput")
    wtpbd = nc.dram_tensor("wtp_b", [F, F], F16, kind="ExternalInput")
    wgbd = nc.dram_tensor("wg_b", [F, F], F16, kind="ExternalInput")
    wtpad = nc.dram_tensor("wtp_a", [F, F], F16, kind="ExternalInput")
    wgad = nc.dram_tensor("wg_a", [F, F], F16, kind="ExternalInput")
    outd = nc.dram_tensor("out", [NB, T, F], F32, kind="ExternalOutput")

    with tile.TileContext(nc) as tc:
        with (
            tc.tile_pool(name="consts", bufs=1) as consts,
            tc.tile_pool(name="npool", bufs=1) as npool,
            tc.tile_pool(name="ntpool", bufs=1) as ntpool,
            tc.tile_pool(name="wres", bufs=2) as wsp,
            tc.tile_pool(name="drains", bufs=1) as drp,
        ):
            u_sb = consts.tile([128, D], F16)
            nc.gpsimd.dma_start(out=u_sb[:], in_=ud[:].to_broadcast((128, D)))
            mb_sb = consts.tile([T, T], F32)
            nc.sync.dma_start(out=mb_sb[:], in_=mbd[:])
            ma_sb = consts.tile([T, T], F32)
            nc.sync.dma_start(out=ma_sb[:], in_=mad[:])
            ident = consts.tile([128, 128], F32)
            nc.sync.dma_start(out=ident[:], in_=identd[:])

            # n (fp32) per batch element; nT (fp16) shared [128, fc, 2*T]
            n_sb = [
                npool.tile([T, F], F32, tag=f"n{b}", name=f"n{b}")
                for b in range(NB)
            ]
            nT = ntpool.tile([128, NFC, NB * T], F16)

            def load_w_half(src, colh):
                """Stream one [2048, 1024] fp16 weight half into a slot."""
                wh = wsp.tile([128, NFC, D], F16, tag="w", name="wh")
                for q in range(7, -1, -1):
                    nc.sync.dma_start(
                        out=wh[:, 2 * q : 2 * (q + 1), :],
                        in_=src[
                            256 * q : 256 * (q + 1), D * colh : D * (colh + 1)
                        ].rearrange("(c p) e -> p c e", p=128),
                    )
                return wh

            # ---------------- Phase A: f, w, g, n, nT ----------------
            with (
                tc.tile_pool(name="featp", bufs=2) as featp,
                tc.tile_pool(name="aw", bufs=2) as awp,
                tc.tile_pool(name="psAt", bufs=2, space="PSUM") as psAt,
            ):
                for b in range(NB):
                    feat = featp.tile([T, NP, D], F16, tag="feat")
                    for q in range(4):
                        nc.sync.dma_start(
                            out=feat[32 * q : 32 * (q + 1)],
                            in_=featd[b, 384 * q : 384 * (q + 1), :].rearrange(
                                "(p c) d -> p c d", c=NP
                            ),
                        )

                    def transp(fc, b=b):
                        pt = psAt.tile([128, 128], F32, tag="pt", name="pt")
                        nc.tensor.transpose(
                            pt[:], n_sb[b][:, 128 * fc : 128 * (fc + 1)],
                            ident[:],
                        )
                        nc.vector.tensor_copy(
                            nT[:, fc, T * b : T * (b + 1)], pt[:]
                        )

                    # f = max_n feat (first, so its transposes start early)
                    facc = awp.tile([T, D], F16, tag="facc")
                    nc.vector.tensor_max(facc[:], feat[:, 0, :], feat[:, 1, :])
                    for c in range(2, NP):
                        nc.vector.tensor_max(facc[:], facc[:], feat[:, c, :])
                    nc.vector.tensor_copy(n_sb[b][:, D:], facc[:])
                    for fc in range(8, NFC):
                        transp(fc)

                    # w[t,n] = sum_d feat*u (fused product+reduce, fp32 accum)
                    wvec = awp.tile([T, NP], F32, tag="wvec")
                    scr = awp.tile([T, D], F16, tag="scr")
                    for c in range(NP):
                        nc.vector.scalar_tensor_tensor(
                            out=scr[:],
                            in0=feat[:, c, :],
                            scalar=1.0,
                            in1=u_sb[:],
                            op0=mult,
                            op1=mult,
                            accum_out=wvec[:, c : c + 1],
                        )
                    # g[t,d] = sum_n w[t,n]*feat[t,n,d], accumulated in n_sb
                    nc.vector.tensor_scalar_mul(
                        n_sb[b][:, :D], feat[:, 0, :], wvec[:, 0:1]
                    )
                    for c in range(1, NP):
                        nc.vector.scalar_tensor_tensor(
                            out=n_sb[b][:, :D],
                            in0=feat[:, c, :],
                            scalar=wvec[:, c : c + 1],
                            in1=n_sb[b][:, :D],
                            op0=mult,
                            op1=add,
                        )
                    for fc in range(8):
                        transp(fc)

            # ---------------- Phases B/C: the two branches ----------------
            for ibr, (wtpd, wgd, mask_sb) in enumerate(
                ((wtpbd, wgbd, mb_sb), (wtpad, wgad, ma_sb))
            ):
                sfx = f"_{ibr}"
                # ---- gb = n @ Wg.T (natural, per b; starts after nT[b]) ----
                gb16 = [
                    drp.tile([T, F], F16, tag=f"gb{b}", name=f"gb{b}{sfx}")
                    for b in range(NB)
                ]
                for colh in range(2):
                    wgh = load_w_half(wgd, colh)
                    with tc.tile_pool(
                        name="ps2", bufs=1, space="PSUM"
                    ) as ps2p:
                        psg = [
                            ps2p.tile(
                                [128, 2, 512], F32, tag=f"psg{b}",
                                name=f"psg{b}{sfx}{colh}",
                            )
                            for b in range(NB)
                        ]
                        for b in range(NB):
                            for i, fc in enumerate(FC_ORDER):
                                for h2 in range(2):
                                    nc.tensor.matmul(
                                        psg[b][:, h2, :],
                                        nT[:, fc, T * b : T * (b + 1)],
                                        wgh[:, fc, 512 * h2 : 512 * (h2 + 1)],
                                        start=(i == 0),
                                        stop=(i == NFC - 1),
                                    )
                            for h2 in range(2):
                                nc.scalar.copy(
                                    gb16[b][
                                        :,
                                        D * colh + 512 * h2 : D * colh
                                        + 512 * (h2 + 1),
                                    ],
                                    psg[b][:, h2, :],
                                )

                # ---- tbT/pbT: [e-sub, proj*echunk, t2] ----
                tp2 = drp.tile([128, 16, 2 * T], F16, tag="tp2",
                               name=f"tp2{sfx}")
                for colh in range(2):  # 0: tb e-cols, 1: pb e-cols
                    wth = load_w_half(wtpd, colh)
                    with tc.tile_pool(
                        name="ps1", bufs=2, space="PSUM"
                    ) as ps1p:
                        for e8 in range(8):
                            p1 = ps1p.tile([128, 2 * T], F32, tag="p1",
                                           name="p1")
                            for i, fc in enumerate(FC_ORDER):
                                nc.tensor.matmul(
                                    p1[:],
                                    wth[:, fc, 128 * e8 : 128 * (e8 + 1)],
                                    nT[:, fc, :],
                                    start=(i == 0),
                                    stop=(i == NFC - 1),
                                )
                            nc.vector.tensor_copy(
                                tp2[:, 8 * colh + e8, :], p1[:]
                            )

                # ---- sbT, mask, out accumulation ----
                with (
                    tc.tile_pool(name="sbp", bufs=2) as sbp,
                    tc.tile_pool(name="ps3", bufs=2, space="PSUM") as ps3p,
                    tc.tile_pool(name="ps4", bufs=2, space="PSUM") as ps4p,
                ):
                    for b in range(NB):
                        psb = ps3p.tile([T, T], F32, tag="psb", name="psb")
                        for ec in range(8):
                            nc.tensor.matmul(
                                psb[:],
                                tp2[:, 8 + ec, T * b : T * (b + 1)],
                                tp2[:, ec, T * b : T * (b + 1)],
                                start=(ec == 0),
                                stop=(ec == 7),
                            )
                        sbm = sbp.tile([T, T], F16, tag="sbm", name="sbm")
                        nc.vector.scalar_tensor_tensor(
                            out=sbm[:],
                            in0=psb[:],
                            scalar=1.0,
                            in1=mask_sb[:],
                            op0=mult,
                            op1=mult,
                        )
                        for h4 in range(4):
                            po = ps4p.tile([T, 512], F32, tag="po", name="po")
                            nc.tensor.matmul(
                                po[:],
                                sbm[:],
                                gb16[b][:, 512 * h4 : 512 * (h4 + 1)],
                                start=True,
                                stop=True,
                            )
                            nc.vector.tensor_add(
                                n_sb[b][:, 512 * h4 : 512 * (h4 + 1)],
                                n_sb[b][:, 512 * h4 : 512 * (h4 + 1)],
                                po[:],
                            )

            for b in range(NB):
                nc.sync.dma_start(out=outd[b], in_=n_sb[b][:])

    nc.compile()
    return nc


def _host_prep(features, Wq, Wtb, Wpb, Wgb, Wta, Wpa, Wga):
    f32 = np.float32
    f16 = np.float16
    feat = np.ascontiguousarray(np.asarray(features, f32)).reshape(B, TN, D)
    u = np.asarray(Wq, f32).sum(axis=0)[None, :]

    def wt(w):  # [e, f] -> [f, e] fp16 contiguous
        return np.ascontiguousarray(np.asarray(w, f32).T.astype(f16))

    wtp_b = np.concatenate([wt(Wtb), wt(Wpb)], axis=1)
    wtp_a = np.concatenate([wt(Wta), wt(Wpa)], axis=1)
    wg_b = wt(Wgb)
    wg_a = wt(Wga)

    idx = np.arange(T)
    maskb = (SCALE * (idx[None, :] > idx[:, None])).astype(f32)  # [j, i]
    maska = (SCALE * (idx[None, :] < idx[:, None])).astype(f32)
    ident = np.eye(128, dtype=f32)

    shared = {
        "u": u.astype(f16),
        "maskb": maskb,
        "maska": maska,
        "ident": ident,
        "wtp_b": wtp_b,
        "wg_b": wg_b,
        "wtp_a": wtp_a,
        "wg_a": wg_a,
    }
    feat16 = feat.astype(f16).reshape(NCORES, NB, TN, D)
    return shared, feat16


def kernel(**inputs) -> np.ndarray:
    if "nc" not in _CACHE:
        _CACHE["nc"] = _build()
    nc = _CACHE["nc"]

    shared, feat16 = _host_prep(**inputs)
    in_maps = [dict(shared, feat=feat16[c]) for c in range(NCORES)]
    res = run_bass_kernel_spmd(nc, in_maps, core_ids=list(range(NCORES)))
    out = np.stack([res.results[c]["out"] for c in range(NCORES)], axis=0)
    return out.reshape(B, T, F).astype(np.float32)
